# revision 1
# baseline (speedup 1.0000x reference)
"""Trainium2 Bass kernel for nn_ChebConvNet (ChebConv K=1 => 3-layer MLP + log_softmax).

Computation per node row (edge_index is inert for K=1 ChebConv):
    h = silu(x @ W0 + b0); h = silu(h @ W1 + b1); h2 = h @ W2 + b2
    out = log_softmax(h2, axis=1)

Strategy: shard the 500k node rows across 8 NeuronCores (row-parallel, no
communication). The ACT engine is the hard bottleneck (2 SiLU passes + exp
at ~0.83ns/elem, no fast modes), so everything is arranged to keep ACT ~100%
busy and every other engine below it:
  - x is transposed and cast to bf16 ON HOST, uploaded as [128, rows]: no PE
    transposes, no DVE transpose copies, and half the x HBM traffic.
  - 2048-row macros: h0/h1 PSUM tiles span 4 banks each (8 total), so each
    SiLU is one [128,2048] ACT op (amortizes the ~185ns per-op access
    latency). mm3 then reuses h1's first two banks for h2 (h1 is dead after
    silu2 reads it), so no third PSUM allocation is needed.
  - mm3 emits row-major h2 via 16 64-wide matmuls with QUAD-row pairing so
    the bf16 output store uses 512B descriptors (below 512B the DMA cost
    model charges 2x).
  - softmax tail (exp -> DVE reduce -> ln -> subtract) runs per 2-macro
    group, software-pipelined one macro behind the MLP: exp/ln slot between
    silu1 and silu2 of a later macro, subtract on GPSIMD, store on SP HWDGE.
  - output is stored bf16 and upcast on host (log_softmax outputs are O(10),
    bf16 keeps rel err ~2e-3, well under the 2e-2 gate).
"""

import math
import numpy as np
import ml_dtypes

import bass_rust
import concourse.bass as bass
import concourse.tile as tile
from concourse import mybir
from concourse.bass_utils import run_bass_kernel_spmd
from concourse.vector_clock import ScopedClock
from bass_rust import add_dep_helper

N_CORES = 8
F_IN = 128
F_HID = 128
F_OUT = 64
MACRO = 2048           # rows per macro (one silu1/silu2 op each)
LOAD_MACROS = 2        # macros per x DMA load
LN_MACROS = 4          # macros per ln batch
BLK = MACRO // 128     # 128-row blocks per macro (16)
H2_PER_MACRO = BLK * F_OUT  # h2 free elems per macro (1024)

_DT = mybir.dt

# this walrus build rejects instructions with more than ONE sync wait; the
# Tile framework freely assigns several. Two patches below: (1) split every
# multi-wait instruction by inserting single-wait NoOp carriers on the same
# engine right before it (order on the engine's sequencer preserves
# semantics); (2) the TileContext tail drain gets the same treatment with
# single-wait drain carriers.
_MAX_DRAIN_WAITS = 1
_N_SPARE_DRAINS = 31

_NOOP_CLS = None
_carrier_counter = [0]


def _noop_cls():
    global _NOOP_CLS
    if _NOOP_CLS is None:
        _NOOP_CLS = getattr(bass_rust, "InstNoOp")
    return _NOOP_CLS


_orig_lower_ordered = tile.TileContext._lower_ordered_insts


def _split_multi_waits(self, ordered):
    cls = _noop_cls()
    new_ordered = {}
    for bb_name, insts in ordered.items():
        new_list = []
        for inst in insts:
            si = inst.sync_info
            waits = list(si.on_wait) if si is not None else []
            if len(waits) > 1:
                for w in waits[:-1]:
                    c = cls(name=f"waitcar-{_carrier_counter[0]}", ins=[],
                            outs=[])
                    _carrier_counter[0] += 1
                    c.engine = inst.engine
                    c.sync_info = bass_rust.SyncInfo(on_wait=[w], on_update=[])
                    new_list.append(c)
                inst.sync_info = bass_rust.SyncInfo(
                    on_wait=[waits[-1]], on_update=list(si.on_update))
            new_list.append(inst)
        new_ordered[bb_name] = new_list
    return _orig_lower_ordered(self, new_ordered)


tile.TileContext._lower_ordered_insts = _split_multi_waits


def _patched_drain_and_barrier(self, tick_clock, wait_clock):
    nc = self.nc
    spare = [nc.sync.drain() for _ in range(_N_SPARE_DRAINS)]
    drain_inst = nc.sync.drain()
    wait_clock.add_sem_waits(
        drain_inst.ins, ScopedClock({None: tick_clock.global_clock})
    )
    si = drain_inst.ins.sync_info
    waits = list(si.on_wait) if si is not None else []
    if len(waits) > _MAX_DRAIN_WAITS:
        chunks = [
            waits[i : i + _MAX_DRAIN_WAITS]
            for i in range(0, len(waits), _MAX_DRAIN_WAITS)
        ]
        head, tail = chunks[:-1], chunks[-1]
        assert len(head) <= _N_SPARE_DRAINS, "bump _N_SPARE_DRAINS"
        for nop_i, chunk in zip(spare, head):
            nop_i.ins.sync_info = bass_rust.SyncInfo(on_wait=chunk, on_update=[])
        drain_inst.ins.sync_info = bass_rust.SyncInfo(
            on_wait=tail, on_update=list(si.on_update)
        )
    nc.all_engine_barrier()
    assert self.sems is not None
    popped = nc._tile_sem_poison_stack.pop()
    assert popped is self._sem_poison
    nc.clear_and_free_semaphores(list(self.sems.allocated().values()))
    nc.all_engine_barrier()


tile.TileContext._drain_and_barrier = _patched_drain_and_barrier


def _build(nc_rows: int, with_b2: bool):
    """Build the per-core Bass module. nc_rows must be a multiple of MACRO."""
    assert nc_rows % MACRO == 0
    n_macros = nc_rows // MACRO
    nc = bass.Bass("TRN2", target_bir_lowering=False, debug=False,
                   num_devices=N_CORES)

    xt_d = nc.dram_tensor("xt", [F_IN, nc_rows], _DT.bfloat16,
                          kind="ExternalInput").ap()
    w0_d = nc.dram_tensor("w0", [F_IN, F_HID], _DT.bfloat16,
                          kind="ExternalInput").ap()
    w1_d = nc.dram_tensor("w1", [F_HID, F_HID], _DT.bfloat16,
                          kind="ExternalInput").ap()
    w2_d = nc.dram_tensor("w2", [F_HID, F_OUT], _DT.bfloat16,
                          kind="ExternalInput").ap()
    b0_d = nc.dram_tensor("b0", [F_HID, 1], _DT.float32,
                          kind="ExternalInput").ap()
    b1_d = nc.dram_tensor("b1", [F_HID, 1], _DT.float32,
                          kind="ExternalInput").ap()
    b2_d = nc.dram_tensor("b2", [1, F_OUT], _DT.bfloat16,
                          kind="ExternalInput").ap()
    out_d = nc.dram_tensor("out", [nc_rows, F_OUT], _DT.bfloat16,
                           kind="ExternalOutput").ap()

    AF = mybir.ActivationFunctionType

    # ln batches: full LN_MACROS groups, but the tail is split so the
    # final group is a single macro (short post-loop critical chain)
    ln_groups = []  # (start_macro, n_macros_in_group)
    mcur = 0
    while n_macros - mcur > LN_MACROS:
        ln_groups.append((mcur, LN_MACROS))
        mcur += LN_MACROS
    rest = n_macros - mcur
    if rest > 1:
        ln_groups.append((mcur, rest - 1))
    ln_groups.append((n_macros - 1, 1))
    grp_of = {}
    for qi, (st, ct) in enumerate(ln_groups):
        for mm in range(st, st + ct):
            grp_of[mm] = qi
    n_lns = len(ln_groups)

    n_loads = (n_macros + LOAD_MACROS - 1) // LOAD_MACROS

    with tile.TileContext(nc) as tc:
        with (
            tc.tile_pool(name="consts", bufs=1) as consts,
            tc.tile_pool(name="xt", bufs=4) as xpool,
            tc.tile_pool(name="h0_ps", bufs=1, space="PSUM") as h0p,
            tc.tile_pool(name="h1x_ps", bufs=1, space="PSUM") as h1xp,
            tc.tile_pool(name="h1y_ps", bufs=1, space="PSUM") as h1yp,
            tc.tile_pool(name="h0_sb", bufs=2) as h0s,
            tc.tile_pool(name="h1x_sb", bufs=2) as h1xs,
            tc.tile_pool(name="h1y_sb", bufs=2) as h1ys,
            tc.tile_pool(name="park", bufs=8) as parkp,
            tc.tile_pool(name="e", bufs=3) as epool,
            tc.tile_pool(name="s", bufs=3) as spool,
            tc.tile_pool(name="lz", bufs=3) as lzpool,
            tc.tile_pool(name="o", bufs=8) as opool,
        ):
            # constants: w0 + b0 + first x tile are on the startup critical
            # path; the rest land during macro 0 compute. HWDGE descriptor
            # generation is serial (~0.6us per dma_start).
            w0 = consts.tile([128, F_HID], _DT.bfloat16, tag="w0")
            nc.sync.dma_start(w0[:], w0_d[:, :])

            LOAD_W = LOAD_MACROS * MACRO
            xt_tiles = {}

            # xt loads ride the SP queue with 3 batches of prefetch so a
            # store blocked on its subtract sem cannot make them late
            def emit_load(c, split=False):
                r0 = c * LOAD_W
                w = min(LOAD_W, nc_rows - r0)
                t = xpool.tile([128, LOAD_W], _DT.bfloat16, tag="xt", name="xt")
                xt_tiles[c] = t
                parts = 2 if (split and w > MACRO) else 1
                half = w // parts
                for pi in range(parts):
                    nc.sync.dma_start(
                        t[:, pi * half:(pi + 1) * half],
                        xt_d[:, r0 + pi * half:r0 + pi * half + half])

            emit_load(0, split=True)

            b0 = consts.tile([128, 1], _DT.float32, tag="b0")
            nc.sync.dma_start(b0[:], b0_d[:, :])
            w1 = consts.tile([128, F_HID], _DT.bfloat16, tag="w1")
            nc.sync.dma_start(w1[:], w1_d[:, :])
            b1 = consts.tile([128, 1], _DT.float32, tag="b1")
            nc.sync.dma_start(b1[:], b1_d[:, :])
            w2 = consts.tile([128, F_OUT], _DT.bfloat16, tag="w2")
            nc.sync.dma_start(w2[:], w2_d[:, :])
            b2 = None
            ones1 = None
            if with_b2:
                b2 = consts.tile([1, F_OUT], _DT.bfloat16, tag="b2")
                nc.sync.dma_start(b2[:], b2_d[:, :])
                ones1 = consts.tile([1, 128], _DT.bfloat16, tag="ones1")
                nc.gpsimd.memset(ones1[:], 1.0)

            for ci in range(1, min(3, n_loads)):
                emit_load(ci)

            # pin ACT/DVE/Pool engine order to emission order (each engine
            # queue is in-order; a mis-scheduled op at the head stalls the
            # stream — the scheduler otherwise reorders freely)
            last_on = {}

            def _order(key, bi):
                if key in last_on:
                    add_dep_helper(bi.ins, last_on[key].ins, sync=False,
                                   reason=f"{key} stream order")
                last_on[key] = bi

            def act_order(bi):
                _order("act", bi)

            def dve_order(bi):
                _order("dve", bi)

            def pool_order(bi):
                _order("pool", bi)

            park_tiles = {}
            e_tiles = {}
            s_tiles = {}
            lz_tiles = {}

            def emit_exp(m):
                e = epool.tile([128, H2_PER_MACRO], _DT.bfloat16,
                               tag="e", name="e")
                e_tiles[m] = e
                act_order(nc.scalar.activation(
                    e[:], park_tiles[m][:], AF.Exp))

            def emit_reduce(m):
                q = grp_of[m]
                mq = m - ln_groups[q][0]
                if mq == 0:
                    s_tiles[q] = spool.tile(
                        [128, LN_MACROS * BLK], _DT.float32, tag="s",
                        name="s")
                off = mq * BLK
                dve_order(nc.vector.tensor_reduce(
                    s_tiles[q][:, off:off + BLK],
                    e_tiles[m][:].rearrange("p (b f) -> p b f", f=F_OUT),
                    axis=mybir.AxisListType.X, op=mybir.AluOpType.add))

            def emit_ln(q):
                nblk = ln_groups[q][1] * BLK
                lz_tiles[q] = lzpool.tile(
                    [128, LN_MACROS * BLK], _DT.float32, tag="lz",
                    name="lz")
                act_order(nc.scalar.activation(
                    lz_tiles[q][:, :nblk], s_tiles[q][:, :nblk], AF.Ln))

            def emit_sub_store(m, engine, half=None):
                q = grp_of[m]
                mq = m - ln_groups[q][0]
                o = opool.tile([128, H2_PER_MACRO], _DT.bfloat16,
                               tag="o", name="o")
                if half is None:
                    blo, bhi = 0, BLK
                else:
                    blo, bhi = half * (BLK // 2), (half + 1) * (BLK // 2)
                off = mq * BLK + blo
                nblk = bhi - blo
                lzb = (lz_tiles[q][:, off:off + nblk]
                       .broadcast_to([128, nblk, F_OUT]))
                bi = engine.tensor_tensor(
                    out=o[:, blo * F_OUT:bhi * F_OUT].rearrange(
                        "p (b f) -> p b f", f=F_OUT),
                    in0=park_tiles[m][:, blo * F_OUT:bhi * F_OUT].rearrange(
                        "p (b f) -> p b f", f=F_OUT),
                    in1=lzb, op=mybir.AluOpType.subtract)
                (dve_order if engine is nc.vector else pool_order)(bi)
                # QUAD-row pairing: block b = 4*P + s holds rows
                # row0 + 512*P + 4*q + s; (s, f) is 512B-contiguous in DRAM
                row0 = m * MACRO + blo * 128
                nrows = nblk * 128
                nc.sync.dma_start(
                    out_d[row0:row0 + nrows, :].rearrange(
                        "(P q s) f -> q P s f", q=128, s=4),
                    o[:, blo * F_OUT:bhi * F_OUT].rearrange(
                        "p (P s f) -> p P s f", s=4, f=F_OUT))

            done_exp = set()
            done_ln = set()
            done_sub = set()
            sub_flip = [0]

            def emit_sub_ready(k):
                # drain pending subtracts whose ln is already emitted —
                # on GPSIMD only (DVE's in-loop queue carries the park
                # copy, which gates the next macro's mm2), max 2 per macro
                n = 0
                for m2 in range(k + 1):
                    if m2 in done_sub or grp_of[m2] not in done_ln:
                        continue
                    emit_sub_store(m2, nc.gpsimd)
                    done_sub.add(m2)
                    n += 1
                    if n == 2:
                        break

            # mm1 for macro k is emitted during macro k-1 (right after the
            # silu2 ops) so it sits ahead of mm3[k-1] in the PE queue and
            # never trails the softmax-tail chain
            h0t = h0p.tile([128, MACRO], _DT.float32, tag="h0t", name="h0t")
            for j in range(MACRO // 512):
                nc.tensor.matmul(
                    h0t[:, j * 512:(j + 1) * 512], lhsT=w0[:],
                    rhs=xt_tiles[0][:, j * 512:(j + 1) * 512],
                    start=True, stop=True)

            for m in range(n_macros):
                c = m // LOAD_MACROS
                if m % LOAD_MACROS == 0 and c + 3 <= n_loads - 1:
                    emit_load(c + 3)

                h1x = h1xp.tile([128, MACRO // 2], _DT.float32, tag="h1x",
                                name="h1x")
                h1y = h1yp.tile([128, MACRO // 2], _DT.float32, tag="h1y",
                                name="h1y")
                h0b = h0s.tile([128, MACRO], _DT.bfloat16, tag="h0b",
                               name="h0b")
                h1bx = h1xs.tile([128, MACRO // 2], _DT.bfloat16, tag="h1bx",
                                 name="h1bx")
                h1by = h1ys.tile([128, MACRO // 2], _DT.bfloat16, tag="h1by",
                                 name="h1by")
                park_tiles[m] = parkp.tile(
                    [128, H2_PER_MACRO], _DT.float32, tag="park", name="park")

                act_order(nc.scalar.activation(
                    h0b[:], h0t[:], AF.Silu, bias=b0[:, 0:1]))

                # ACT slot between silu1 and the silu2 pair: the pipelined
                # exp of macro m-2 (inputs long ready) keeps ACT busy while
                # the h1y release chain (mm3[m-1] -> park copies -> mm2[m])
                # completes; at the final macro also drain exp[m-1]
                if m == 1:
                    emit_exp(0)
                    emit_reduce(0)
                    done_exp.add(0)
                if m >= 2 and (m - 2) not in done_exp:
                    emit_exp(m - 2)
                    emit_reduce(m - 2)
                    done_exp.add(m - 2)
                if m == n_macros - 1 and m >= 1 and (m - 1) not in done_exp:
                    emit_exp(m - 1)
                    emit_reduce(m - 1)
                    done_exp.add(m - 1)
                for q in range(n_lns):
                    if q in done_ln:
                        continue
                    e_last = ln_groups[q][0] + ln_groups[q][1] - 1
                    # one extra macro of slack so ln never waits its reduce
                    if e_last <= m - 3:
                        emit_ln(q)
                        done_ln.add(q)
                    break

                # mm2: x-half (banks untouched by the tail) first, then the
                # y-half whose PSUM tile waits on the previous macro's park
                # copies
                for j in range(2):
                    nc.tensor.matmul(
                        h1x[:, j * 512:(j + 1) * 512], lhsT=w1[:],
                        rhs=h0b[:, j * 512:(j + 1) * 512],
                        start=True, stop=True)
                for j in range(2):
                    nc.tensor.matmul(
                        h1y[:, j * 512:(j + 1) * 512], lhsT=w1[:],
                        rhs=h0b[:, (j + 2) * 512:(j + 3) * 512],
                        start=True, stop=True)

                # silu2 as two 1024-wide ops: y first (its mm3 blocks and
                # park copies form the critical chain), then x
                act_order(nc.scalar.activation(
                    h1by[:], h1y[:], AF.Silu, bias=b1[:, 0:1]))
                act_order(nc.scalar.activation(
                    h1bx[:], h1x[:], AF.Silu, bias=b1[:, 0:1]))

                # next macro's mm1 (PE queue position: before mm3[m])
                if m + 1 < n_macros:
                    c1 = (m + 1) // LOAD_MACROS
                    xoff = ((m + 1) % LOAD_MACROS) * MACRO
                    h0t = h0p.tile([128, MACRO], _DT.float32, tag="h0t",
                                   name="h0t")
                    for j in range(MACRO // 512):
                        nc.tensor.matmul(
                            h0t[:, j * 512:(j + 1) * 512], lhsT=w0[:],
                            rhs=xt_tiles[c1][:, xoff + j * 512:
                                             xoff + (j + 1) * 512],
                            start=True, stop=True)

                # mm3: h2 lives in h1y's two PSUM banks (dead after silu2y
                # reads them). Blocks 8-15 read h1by (ready first), blocks
                # 0-7 read h1bx. One accumulation group per PSUM bank
                # (start zeroes the whole 2KB bank region; disjoint block
                # writes clear it). QUAD-row pairing for 512B store descs.
                h2v = h1y[:]
                n_per_bank = 512 // F_OUT  # blocks per PSUM bank (8)
                for half in (1, 0):
                    src_b = h1by if half else h1bx
                    for ib in range(n_per_bank):
                        b = half * n_per_bank + ib
                        P, s = b // 4, b % 4
                        Ploc = P % 2
                        lview = (src_b[:, Ploc * 512:(Ploc + 1) * 512]
                                 .rearrange("p (q four) -> p q four", four=4)
                                 [:, :, s])
                        is_last = (ib == n_per_bank - 1) and not with_b2
                        nc.tensor.matmul(
                            h2v[:, b * F_OUT:(b + 1) * F_OUT],
                            lhsT=lview, rhs=w2[:],
                            start=(ib == 0), stop=is_last)
                    if with_b2:
                        for ib in range(n_per_bank):
                            b = half * n_per_bank + ib
                            nc.tensor.matmul(
                                h2v[:, b * F_OUT:(b + 1) * F_OUT],
                                lhsT=ones1[:], rhs=b2[:],
                                start=False, stop=(ib == n_per_bank - 1))

                # park h2 (GPSIMD cannot read PSUM, so this is one DVE
                # copy); it releases h1y for the next macro's mm2 y-half
                dve_order(nc.vector.tensor_copy(park_tiles[m][:], h2v[:]))

                emit_sub_ready(m)

            # flush, in dependency order. After the loop: exp/reduce done
            # for all but the last macro; ln done through group n_lns-3.
            # ACT chain: ln[second-last], exp[last], ln[last];
            # DVE chain: copy[last] (already), subs, reduce[last], sub-half;
            # Pool: remaining subs. Stores trail each sub on SP.
            mL = n_macros - 1
            qL = n_lns - 1
            if qL - 1 >= 0 and (qL - 1) not in done_ln:
                emit_ln(qL - 1)
                done_ln.add(qL - 1)
            pend = [m for m in range(n_macros)
                    if m not in done_sub and m != mL]
            # oldest pending subs go first; alternate Pool/DVE (Pool is
            # slower per op, DVE slots into its idle window)
            for i, m2 in enumerate(pend):
                emit_sub_store(m2, nc.gpsimd if i % 2 == 0 else nc.vector)
                done_sub.add(m2)
            if mL not in done_exp:
                emit_exp(mL)
                emit_reduce(mL)
                done_exp.add(mL)
            if qL not in done_ln:
                emit_ln(qL)
                done_ln.add(qL)
            emit_sub_store(mL, nc.vector, half=0)
            emit_sub_store(mL, nc.gpsimd, half=1)
            done_sub.add(mL)

    return nc


_BUILD_CACHE = {}


def _get_module(nc_rows: int, with_b2: bool):
    key = (nc_rows, with_b2)
    if key not in _BUILD_CACHE:
        _BUILD_CACHE[key] = _build(nc_rows, with_b2)
    return _BUILD_CACHE[key]


def _prepare(x, W0, b0, W1, b1, W2, b2):
    """Host-side preprocessing shared by kernel() and test harnesses.

    Returns (nc, in_maps, n, per): the built module, per-core input maps,
    the true row count and the padded per-core row count.
    """
    x = np.ascontiguousarray(np.asarray(x), dtype=np.float32)
    n = x.shape[0]
    per = int(math.ceil(n / N_CORES / MACRO)) * MACRO
    total = per * N_CORES

    bf = ml_dtypes.bfloat16
    xp = np.zeros((total, F_IN), dtype=bf)
    xp[:n] = x.astype(bf)
    # per-core feature-major (transposed) layout: [8, 128, per]
    xt = np.ascontiguousarray(xp.reshape(N_CORES, per, F_IN).transpose(0, 2, 1))

    w0b = np.ascontiguousarray(np.asarray(W0, dtype=np.float32)).astype(bf)
    w1b = np.ascontiguousarray(np.asarray(W1, dtype=np.float32)).astype(bf)
    w2b = np.ascontiguousarray(np.asarray(W2, dtype=np.float32)).astype(bf)
    b0f = np.asarray(b0, dtype=np.float32).reshape(F_HID, 1)
    b1f = np.asarray(b1, dtype=np.float32).reshape(F_HID, 1)
    b2f = np.asarray(b2, dtype=np.float32).reshape(1, F_OUT)
    with_b2 = bool(np.any(b2f))
    b2b = b2f.astype(bf)

    nc = _get_module(per, with_b2)

    in_maps = []
    for i in range(N_CORES):
        in_maps.append({
            "xt": xt[i],
            "w0": w0b, "w1": w1b, "w2": w2b,
            "b0": b0f, "b1": b1f, "b2": b2b,
        })
    return nc, in_maps, n, per


def kernel(x, edge_index=None, W0=None, b0=None, W1=None, b1=None, W2=None,
           b2=None, **_unused):
    nc, in_maps, n, per = _prepare(x, W0, b0, W1, b1, W2, b2)
    res = run_bass_kernel_spmd(nc, in_maps, list(range(N_CORES)))
    out = np.concatenate([res.results[i]["out"] for i in range(N_CORES)],
                         axis=0)
    return np.ascontiguousarray(out[:n]).astype(np.float32)

